# revision 1
# baseline (speedup 1.0000x reference)
"""MultiHeadSemGConv Trainium2 kernel.

Computes, for x:[B,N,CIN], W:[H,2,CIN,HC], e:[H,N*K], bias:[H,HC],
rows/cols:[N*K] (int32 edge list):

    h = einsum('bnc,hscd->shbnd', x, W)             # two projections per head
    A = softmax(scatter(e at (rows,cols), NEG))     # [H,N,N]
    out[h,b] = diag(A)*h0 + (A - diag)@h1 + bias    # -> [B,N,H*HC]

Strategy: pure data-parallel over batch across 8 NeuronCores.  The tiny
[H,98,98] adjacency softmax is precomputed on host; the heavy lifting
(x projection + graph mixing over 100MB of activations) runs on device:

  per core (128 samples):
    - DMA x in flat 128-partition tiles, casting f32->fp16 in the DMA
    - PE transpose (matmul with identity) -> xT chunk tiles
      [c(2x128), 16*98+30 cols] fp16 in SBUF (30-col overlap keeps every
      per-sample phase-1 stationary at m=128)
    - phase 1, per sample b: h[128,512] = xT[:, 98b:98b+128].T @ Wall
      (2 accumulating fp16 matmuls, f32 PSUM), 2 samples per PSUM tile
    - phase 2, per 8-sample group, per head: 2 accumulating matmuls with
      host-built graph matrices (diag-embed & A_off^T, zero-padded to
      K=128); bias added during the PSUM->SBUF copy on DVE
    - DMA out f32
"""

import os
import sys

import numpy as np

try:
    import concourse.bass as bass  # noqa: F401
except Exception:  # pragma: no cover - fresh grading dir fallback
    for p in ("/opt/trn_rl_repo", "/root/.axon_site/_ro/trn_rl_repo"):
        if os.path.isdir(p) and p not in sys.path:
            sys.path.insert(0, p)
    import concourse.bass as bass  # noqa: F401

# ---------------------------------------------------------------- constants
NLM = 98          # landmarks (graph nodes)
HEADS = 4
CIN = 256
HC = 64
HD = 512          # h width = 2 (s) * 4 (heads) * 64 (d)
B = 1024
NCORES = 8
NS = B // NCORES  # samples per core = 128
P = 128
G = 8             # samples per output group
NGRP = NS // G    # 16 groups per core
OVL = 30          # overlap cols so every phase-1 lhsT can be m=128
NEG = -9e15

CHS = 16                    # samples per xT chunk
NCH = NS // CHS             # 8 chunks
CHW = CHS * NLM             # 1568 cols per chunk (+OVL)
NFT = NS * NLM // P         # 98 flat 128-row tiles
NPAIR = NFT // 2            # 49 transpose pairs
DGF = 14                    # flat tiles per input DMA group
NDG = NFT // DGF            # 7 DMA groups

_CACHE = {}


def _build_nc():
    import concourse.mybir as mybir
    import concourse.tile as tile
    from concourse import bacc

    f16 = mybir.dt.float16
    f32 = mybir.dt.float32

    nc = bacc.Bacc(None, target_bir_lowering=False)

    x = nc.dram_tensor("x", [NS * NLM, CIN], f32, kind="ExternalInput")
    wall = nc.dram_tensor("wall", [P, 2, HD], f16, kind="ExternalInput")
    gmat = nc.dram_tensor("gmat", [P, 2 * HEADS * P], f16, kind="ExternalInput")
    biast = nc.dram_tensor("biast", [NLM, G * 256], f32, kind="ExternalInput")
    ident = nc.dram_tensor("ident", [P, P], f16, kind="ExternalInput")
    out = nc.dram_tensor("out", [NS * NLM, CIN], f32, kind="ExternalOutput")

    with tile.TileContext(nc) as tc:
        with (
            tc.tile_pool(name="const", bufs=1) as constp,
            tc.tile_pool(name="xin", bufs=4) as xinp,
            tc.tile_pool(name="xt", bufs=1) as xtp,
            tc.tile_pool(name="hgrp", bufs=2) as hgp,
            tc.tile_pool(name="osb", bufs=2) as osbp,
            tc.tile_pool(name="ptr", bufs=2, space="PSUM") as ptrp,
            tc.tile_pool(name="phs", bufs=2, space="PSUM") as phsp,
            tc.tile_pool(name="pout", bufs=2, space="PSUM") as poutp,
        ):
            ident_sb = constp.tile([P, P], f16, tag="ident")
            nc.sync.dma_start(ident_sb[:], ident[:])
            ident32_sb = constp.tile([P, P], f32, tag="ident32")
            nc.vector.tensor_copy(ident32_sb[:], ident_sb[:])
            wall_sb = constp.tile([P, 2, HD], f16, tag="wall")
            nc.sync.dma_start(wall_sb[:], wall[:])
            gm_sb = constp.tile([P, 2 * HEADS * P], f16, tag="gmat")
            nc.sync.dma_start(gm_sb[:], gmat[:])
            bias_sb = constp.tile([NLM, G * 256], f32, tag="biast")
            nc.sync.dma_start(bias_sb[:], biast[:])

            xt = [
                xtp.tile([P, 2, CHW + OVL], f16, tag=f"xt{k}", name=f"xt{k}")
                for k in range(NCH)
            ]
            nc.vector.memset(xt[NCH - 1][:, :, CHW:], 0.0)

            bias3 = bias_sb[:].rearrange("p (s c) -> p s c", s=G)

            def route_piece(g0, ptr, off, w):
                """Copy ptr[:, :, off:off+w] (global xT cols [g0,g0+w)) into
                the chunk tiles, including overlap duplication."""
                while w > 0:
                    k = g0 // CHW
                    lo = g0 - k * CHW
                    pw = min(w, CHW - lo)
                    nc.scalar.copy(
                        out=xt[k][:, :, lo : lo + pw],
                        in_=ptr[:, :, off : off + pw],
                    )
                    # overlap region of the previous chunk
                    if k > 0 and lo < OVL:
                        ow = min(pw, OVL - lo)
                        nc.scalar.copy(
                            out=xt[k - 1][:, :, CHW + lo : CHW + lo + ow],
                            in_=ptr[:, :, off : off + ow],
                        )
                    g0 += pw
                    off += pw
                    w -= pw

            def transpose_pair(xin_ap, gft):
                """Transpose 2 flat tiles (xin_ap: [P, 2, CIN] fp16) whose
                first global flat-tile index is gft."""
                ptr = ptrp.tile([P, 2, 2 * P], mybir.dt.float32, tag="ptr")
                ident_ap = (
                    ident32_sb[:]
                    if xin_ap.dtype == mybir.dt.float32
                    else ident_sb[:]
                )
                for a in range(2):
                    for cc in range(2):
                        nc.tensor.matmul(
                            ptr[:, cc, a * P : (a + 1) * P],
                            xin_ap[:, a, cc * P : (cc + 1) * P],
                            ident_ap,
                            start=True,
                            stop=True,
                        )
                route_piece(gft * P, ptr, 0, 2 * P)

            def emit_a_group(dg, split=False):
                """DMA DGF flat x tiles (cast to fp16) and transpose them.
                With split=True use per-pair DMAs so the PE can start as soon
                as the first 256 rows land (cuts kernel-head latency)."""
                base = dg * DGF * P
                if split:
                    # HWDGE f32 loads (no descriptor-gen serialization on the
                    # GpSimd queue) + fp32 transposes; only DGF tiles pay the
                    # 2x fp32 PE rate, and the kernel head shrinks.
                    for pr in range(DGF // 2):
                        xs = xinp.tile([P, 2, CIN], f32, tag="xin0")
                        b0 = base + pr * 2 * P
                        nc.sync.dma_start(
                            xs[:],
                            x[b0 : b0 + 2 * P, :].rearrange(
                                "(t p) c -> p t c", p=P
                            ),
                        )
                        transpose_pair(xs[:], dg * DGF + pr * 2)
                    return
                xin = xinp.tile([P, DGF, CIN], f16, tag="xin")
                nc.gpsimd.dma_start(
                    xin[:],
                    x[base : base + DGF * P, :].rearrange("(t p) c -> p t c", p=P),
                )
                for pr in range(DGF // 2):
                    transpose_pair(
                        xin[:, pr * 2 : pr * 2 + 2, :], dg * DGF + pr * 2
                    )

            def emit_b_group(gi):
                """Phase 1 for G samples, then phase 2 + bias + store."""
                hgrp = hgp.tile([P, G * HD], f16, tag="hgrp")
                ck = (gi * G) // CHS
                for pi in range(G // 2):
                    hps = phsp.tile([P, 2, HD], mybir.dt.float32, tag="hps")
                    for a in range(2):
                        b = gi * G + pi * 2 + a
                        lb = b - ck * CHS
                        for cc in range(2):
                            nc.tensor.matmul(
                                hps[:, a, :],
                                xt[ck][:, cc, NLM * lb : NLM * lb + P],
                                wall_sb[:, cc, :],
                                start=(cc == 0),
                                stop=(cc == 1),
                            )
                    dst = hgrp[:, pi * 2 * HD : (pi + 1) * 2 * HD].rearrange(
                        "p (a f) -> p a f", a=2
                    )
                    if (gi * G // 2 + pi) % 5 < 3:
                        nc.scalar.copy(out=dst, in_=hps[:])
                    else:
                        nc.vector.tensor_copy(dst, hps[:])

                hg3 = hgrp[:].rearrange("p (s f) -> p s f", s=G)
                osb = osbp.tile([NLM, G * 256], mybir.dt.float32, tag="osb")
                osb3 = osb[:].rearrange("p (s c) -> p s c", s=G)
                for hd in range(HEADS):
                    pouts = poutp.tile([P, G * HC], mybir.dt.float32, tag="pout")
                    po3 = pouts[:].rearrange("p (s f) -> p s f", s=G)
                    for prt in range(2):
                        q = hd * 2 + prt
                        nc.tensor.matmul(
                            po3,
                            gm_sb[:, q * P : (q + 1) * P],
                            hg3[:, :, prt * 256 + hd * HC : prt * 256 + (hd + 1) * HC],
                            start=(prt == 0),
                            stop=(prt == 1),
                        )
                    nc.vector.tensor_add(
                        out=osb3[:, :, hd * HC : (hd + 1) * HC],
                        in0=po3[:NLM],
                        in1=bias3[:, :, hd * HC : (hd + 1) * HC],
                    )
                ov = out[gi * G * NLM : (gi + 1) * G * NLM, :].rearrange(
                    "(s i) c -> i s c", s=G
                )
                if gi < NGRP - 1:
                    nc.sync.dma_start(ov, osb3)
                else:
                    # split the final store so the kernel tail is shorter
                    h = G // 2
                    nc.sync.dma_start(ov[:, :h], osb3[:, :h])
                    nc.sync.dma_start(ov[:, h:], osb3[:, h:])

            # chunk k is fully transposed once DMA group ceil((1598+1568k)/1792)
            # has been processed; interleave A and B so PE never starves.
            ready_dg = [
                -(-(CHW * k + CHW + OVL) // (DGF * P)) for k in range(NCH)
            ]  # per chunk, 1-indexed count of A groups needed
            ready_dg[NCH - 1] = NDG
            # Prefetch input DMAs two B-groups ahead: the SWDGE descriptor
            # generation shares the GpSimd FIFO with the bias adds, so a
            # just-in-time DMA would queue behind a ~4.5us add and starve PE.
            emitted = 0
            for gi in range(NGRP):
                need = ready_dg[(min(gi + 2, NGRP - 1) * G) // CHS]
                while emitted < need:
                    emit_a_group(emitted, split=(emitted == 0))
                    emitted += 1
                emit_b_group(gi)

    nc.compile()
    return nc


def _host_prep(W, e, bias, rows, cols):
    """Precompute fp16 device constants from the small parameter tensors."""
    W = np.asarray(W, np.float32)
    e = np.asarray(e, np.float32)
    bias = np.asarray(bias, np.float32)
    rows = np.asarray(rows, np.int64)
    cols = np.asarray(cols, np.int64)

    logits = np.full((HEADS, NLM, NLM), NEG, np.float64)
    logits[:, rows, cols] = e.astype(np.float64)
    m = logits.max(axis=-1, keepdims=True)
    p = np.exp(logits - m)
    A = p / p.sum(axis=-1, keepdims=True)            # [H, N, N]
    dg = np.einsum("hii->hi", A).copy()              # [H, N]
    A_off = A.copy()
    np.einsum("hii->hi", A_off)[:] = 0.0

    # Wall: [c, (s, h, d)] -> chunked [128, 2, 512]
    wr = W.transpose(2, 1, 0, 3).reshape(CIN, 2 * HEADS * HC)   # [c, shd]
    wall = np.ascontiguousarray(
        wr.reshape(2, P, 2 * HEADS * HC).transpose(1, 0, 2)
    ).astype(np.float16)

    # graph matrices, zero-padded to 128 rows & cols: [j, (head, part, i)]
    gm = np.zeros((P, HEADS, 2, P), np.float32)
    idx = np.arange(NLM)
    for h in range(HEADS):
        gm[idx, h, 0, idx] = dg[h]
        gm[:NLM, h, 1, :NLM] = A_off[h].T
    gmat = np.ascontiguousarray(gm.reshape(P, 2 * HEADS * P)).astype(np.float16)

    bcat = bias.reshape(HEADS * HC)                  # col = h*64+d
    biast = np.ascontiguousarray(np.tile(bcat, (NLM, G))).astype(np.float32)

    ident = np.eye(P, dtype=np.float16)
    return {"wall": wall, "gmat": gmat, "biast": biast, "ident": ident}


def kernel(x, W, e, bias, rows, cols):
    from concourse.bass_utils import run_bass_kernel_spmd

    if "nc" not in _CACHE:
        _CACHE["nc"] = _build_nc()
    nc = _CACHE["nc"]

    consts = _host_prep(W, e, bias, rows, cols)
    x = np.ascontiguousarray(np.asarray(x, np.float32)).reshape(B, NLM, CIN)

    in_maps = []
    for ci in range(NCORES):
        shard = np.ascontiguousarray(
            x[ci * NS : (ci + 1) * NS].reshape(NS * NLM, CIN)
        )
        in_maps.append({"x": shard, **consts})

    res = run_bass_kernel_spmd(
        nc,
        in_maps,
        core_ids=list(range(NCORES)),
        trace=bool(int(os.environ.get("KERNEL_TRACE", "0"))),
    )
    _CACHE["last_results"] = res

    out = np.concatenate(
        [r["out"].reshape(NS, NLM, HEADS * HC) for r in res.results], axis=0
    )
    return out



# revision 7
# speedup vs baseline: 1.1552x; 1.1552x over previous
"""MultiHeadSemGConv Trainium2 kernel.

Computes, for x:[B,N,CIN], W:[H,2,CIN,HC], e:[H,N*K], bias:[H,HC],
rows/cols:[N*K] (int32 edge list):

    h = einsum('bnc,hscd->shbnd', x, W)             # two projections per head
    A = softmax(scatter(e at (rows,cols), NEG))     # [H,N,N]
    out[h,b] = diag(A)*h0 + (A - diag)@h1 + bias    # -> [B,N,H*HC]

Strategy: pure data-parallel over batch across 8 NeuronCores.  The tiny
[H,98,98] adjacency softmax is precomputed on host; the heavy lifting
(x projection + graph mixing over 100MB of activations) runs on device:

  per core (128 samples):
    - DMA x in flat 128-partition tiles (group 0: f32 halves on the two
      HWDGE queues + DVE cast; later groups: fp16-casting SWDGE bulk)
    - PE transpose (matmul with identity) -> xT chunk tiles
      [c(2x128), 16*98+30 cols] fp16 in SBUF (30-col overlap keeps every
      per-sample phase-1 stationary at m=128)
    - phase 1, per sample b: h[128,512] = xT[:, 98b:98b+128].T @ Wall
      (2 accumulating fp16 matmuls, f32 PSUM), 2 samples per PSUM tile;
      rows [:98] copied into one of two persistent h tiles whose row 98
      holds the bias pattern
    - phase 2, per 8-sample group, per head: ONE matmul with the
      host-built A_off^T (contract k=99: 98 nodes + bias row), then a
      fused DVE op  out = dg (.) h0 + psum  adds the diagonal part.
      Phase 2 of group g is interleaved into phase 1 of group g+1.
    - DMA out f32
"""

import os
import sys

import numpy as np

try:
    import concourse.bass as bass  # noqa: F401
except Exception:  # pragma: no cover - fresh grading dir fallback
    for p in ("/opt/trn_rl_repo", "/root/.axon_site/_ro/trn_rl_repo"):
        if os.path.isdir(p) and p not in sys.path:
            sys.path.insert(0, p)
    import concourse.bass as bass  # noqa: F401

# ---------------------------------------------------------------- constants
NLM = 98          # landmarks (graph nodes)
HEADS = 4
CIN = 256
HC = 64
HD = 512          # h width = 2 (s) * 4 (heads) * 64 (d)
B = 1024
NCORES = 8
NS = B // NCORES  # samples per core = 128
P = 128
G = 8             # samples per output group
NGRP = NS // G    # 16 groups per core
OVL = 30          # overlap cols so every phase-1 lhsT can be m=128
NEG = -9e15

CHS = 16                    # samples per xT chunk
NCH = NS // CHS             # 8 chunks
CHW = CHS * NLM             # 1568 cols per chunk (+OVL)
NFT = NS * NLM // P         # 98 flat 128-row tiles
NPAIR = NFT // 2            # 49 transpose pairs
DGF = 14                    # flat tiles per input DMA group
NDG = NFT // DGF            # 7 DMA groups
SPL = 7                     # pairs in the split (head) group 0

_CACHE = {}


def _build_nc():
    import concourse.mybir as mybir
    import concourse.tile as tile
    from concourse import bacc

    f16 = mybir.dt.float16
    f32 = mybir.dt.float32
    MUL = mybir.AluOpType.mult
    ADD = mybir.AluOpType.add

    nc = bacc.Bacc(None, target_bir_lowering=False)

    x = nc.dram_tensor("x", [NS * NLM, CIN], f32, kind="ExternalInput")
    wall = nc.dram_tensor("wall", [P, 2, HD], f16, kind="ExternalInput")
    gmat = nc.dram_tensor("gmat", [P, HEADS * P], f16, kind="ExternalInput")
    dgvt = nc.dram_tensor("dgvt", [NLM, HEADS], f32, kind="ExternalInput")
    brow = nc.dram_tensor("brow", [1, G * HD], f16, kind="ExternalInput")
    ident = nc.dram_tensor("ident", [P, P], f16, kind="ExternalInput")
    out = nc.dram_tensor("out", [NS * NLM, CIN], f32, kind="ExternalOutput")

    with tile.TileContext(nc) as tc:
        with (
            tc.tile_pool(name="const", bufs=1) as constp,
            tc.tile_pool(name="x0", bufs=1) as x0p,
            tc.tile_pool(name="xf", bufs=2) as xfp,
            tc.tile_pool(name="xin", bufs=4) as xinp,
            tc.tile_pool(name="xt", bufs=1) as xtp,
            tc.tile_pool(name="hg", bufs=1) as hgp,
            tc.tile_pool(name="osb", bufs=2) as osbp,
            tc.tile_pool(name="ptr", bufs=2, space="PSUM") as ptrp,
            tc.tile_pool(name="phs", bufs=2, space="PSUM") as phsp,
            tc.tile_pool(name="pout", bufs=2, space="PSUM") as poutp,
        ):
            # ---- head DMAs: small consts + split group-0 x loads --------
            dgv_sb = constp.tile([NLM, HEADS], f32, tag="dgv")
            nc.scalar.dma_start(dgv_sb[:], dgvt[:])
            ident_sb = constp.tile([P, P], f16, tag="ident")
            nc.sync.dma_start(ident_sb[:], ident[:])

            xs32 = []
            for pr in range(SPL):
                xs = x0p.tile([P, 2, CIN], f32, tag=f"xs{pr}", name=f"xs{pr}")
                xs32.append(xs)
            wall_sb = constp.tile([P, 2, HD], f16, tag="wall")
            gm_sb = constp.tile([P, HEADS * P], f16, tag="gmat")

            def pair_dma(pr):
                b0 = pr * 2 * P
                src = x[b0 : b0 + 2 * P, :].rearrange("(t p) c -> p t c", p=P)
                nc.sync.dma_start(xs32[pr][:, :, 0:128], src[:, :, 0:128])
                nc.scalar.dma_start(xs32[pr][:, :, 128:256], src[:, :, 128:256])

            for pr in range(4):
                pair_dma(pr)
            nc.sync.dma_start(wall_sb[:, :, 0:256], wall[:, :, 0:256])
            nc.scalar.dma_start(wall_sb[:, :, 256:512], wall[:, :, 256:512])

            hgt = [
                hgp.tile([P, G * HD], f16, tag=f"hg{i}", name=f"hg{i}")
                for i in range(2)
            ]

            xt = [
                xtp.tile([P, 2, CHW + OVL], f16, tag=f"xt{k}", name=f"xt{k}")
                for k in range(NCH)
            ]
            nc.vector.memset(xt[NCH - 1][:, :, CHW:], 0.0)

            def route_piece(g0, ptr, off, w):
                """Copy ptr[:, :, off:off+w] (global xT cols [g0,g0+w)) into
                the chunk tiles, including overlap duplication."""
                while w > 0:
                    k = g0 // CHW
                    lo = g0 - k * CHW
                    pw = min(w, CHW - lo)
                    nc.scalar.copy(
                        out=xt[k][:, :, lo : lo + pw],
                        in_=ptr[:, :, off : off + pw],
                    )
                    # overlap region of the previous chunk
                    if k > 0 and lo < OVL:
                        ow = min(pw, OVL - lo)
                        nc.scalar.copy(
                            out=xt[k - 1][:, :, CHW + lo : CHW + lo + ow],
                            in_=ptr[:, :, off : off + ow],
                        )
                    g0 += pw
                    off += pw
                    w -= pw

            def transpose_pair(xin_ap, gft):
                """Transpose 2 flat tiles (xin_ap: [P, 2, CIN] fp16) whose
                first global flat-tile index is gft."""
                ptr = ptrp.tile([P, 2, 2 * P], f32, tag="ptr")
                for a in range(2):
                    for cc in range(2):
                        nc.tensor.matmul(
                            ptr[:, cc, a * P : (a + 1) * P],
                            xin_ap[:, a, cc * P : (cc + 1) * P],
                            ident_sb[:],
                            start=True,
                            stop=True,
                        )
                route_piece(gft * P, ptr, 0, 2 * P)

            def split_pair_T(pr):
                """Cast one head-group pair f32->fp16 on DVE, then PE
                transpose + route."""
                xf = xfp.tile([P, 2, CIN], f16, tag="xf")
                nc.vector.tensor_copy(xf[:], xs32[pr][:])
                transpose_pair(xf[:], pr * 2)

            # ---- bulk input: fp16-casting SWDGE, DGF flat tiles a shot --
            xin_tiles = {}

            def emit_a_dma(dg):
                xin = xinp.tile([P, DGF, CIN], f16, tag="xin")
                base = dg * DGF * P
                nc.gpsimd.dma_start(
                    xin[:],
                    x[base : base + DGF * P, :].rearrange("(t p) c -> p t c", p=P),
                )
                xin_tiles[dg] = xin

            def emit_pair_T(tp):
                """Transpose bulk pair tp (pairs SPL.. come from bulk)."""
                ft = tp * 2
                dg, pr = divmod(ft - 0, DGF)
                xin = xin_tiles[dg]
                transpose_pair(xin[:, pr : pr + 2, :], ft)

            hg3s = [h[:].rearrange("p (s f) -> p s f", s=G) for h in hgt]

            def emit_p2_head(gi, hd, osb3):
                """Phase 2 for one head of group gi: one k=99 matmul
                (A_off^T + bias row), then fused  out = dg (.) h0 + psum."""
                hg3 = hg3s[gi % 2]
                pouts = poutp.tile([P, G * HC], f32, tag="pout")
                po3 = pouts[:].rearrange("p (s f) -> p s f", s=G)
                nc.tensor.matmul(
                    po3,
                    gm_sb[0:99, hd * P : (hd + 1) * P],
                    hg3[0:99, :, 256 + hd * HC : 256 + (hd + 1) * HC],
                    start=True,
                    stop=True,
                )
                nc.vector.scalar_tensor_tensor(
                    out=osb3[:, :, hd * HC : (hd + 1) * HC],
                    in0=hg3[0:98, :, hd * HC : (hd + 1) * HC],
                    scalar=dgv_sb[:, hd : hd + 1],
                    in1=po3[0:98],
                    op0=MUL,
                    op1=ADD,
                )

            def store(gi, osb3, s0=0, s1=G):
                ov = out[gi * G * NLM : (gi + 1) * G * NLM, :].rearrange(
                    "(s i) c -> i s c", s=G
                )
                nc.sync.dma_start(ov[:, s0:s1], osb3[:, s0:s1])

            osb_t = {}

            def emit_b_phase1(gi, prev):
                """Phase 1 for G samples of gi; phase 2 of group `prev`
                interleaved between the pairs."""
                hgrp = hgt[gi % 2]
                if prev is not None:
                    osb = osbp.tile([NLM, G * 256], f32, tag="osb")
                    osb3 = osb[:].rearrange("p (s c) -> p s c", s=G)
                    osb_t[prev] = osb3
                ck = gi // 2
                for pi in range(G // 2):
                    hps = phsp.tile([P, 2, HD], f32, tag="hps")
                    for a in range(2):
                        b = gi * G + pi * 2 + a
                        lb = b - ck * CHS
                        for cc in range(2):
                            nc.tensor.matmul(
                                hps[:, a, :],
                                xt[ck][:, cc, NLM * lb : NLM * lb + P],
                                wall_sb[:, cc, :],
                                start=(cc == 0),
                                stop=(cc == 1),
                            )
                    dst = hgrp[0:98, pi * 2 * HD : (pi + 1) * 2 * HD].rearrange(
                        "p (a f) -> p a f", a=2
                    )
                    if pi == 0:
                        nc.vector.tensor_copy(dst, hps[0:98])
                    else:
                        nc.scalar.copy(out=dst, in_=hps[0:98])
                    if prev is not None:
                        if pi < 3:
                            emit_p2_head(prev, pi, osb_t[prev])
                        else:
                            emit_p2_head(prev, 3, osb_t[prev])
                            store(prev, osb_t[prev])

            def emit_p2_flush(gi):
                """Phase 2 for the final group, split in half-groups with
                split stores for a short kernel tail."""
                osb = osbp.tile([NLM, G * 256], f32, tag="osb")
                osb3 = osb[:].rearrange("p (s c) -> p s c", s=G)
                hg3 = hg3s[gi % 2]
                for half in range(2):
                    s0, s1 = half * 4, half * 4 + 4
                    for hd in range(HEADS):
                        pouts = poutp.tile([P, G * HC], f32, tag="pout")
                        po3 = pouts[:].rearrange("p (s f) -> p s f", s=G)
                        nc.tensor.matmul(
                            po3[:, s0:s1, :],
                            gm_sb[0:99, hd * P : (hd + 1) * P],
                            hg3[0:99, s0:s1, 256 + hd * HC : 256 + (hd + 1) * HC],
                            start=True,
                            stop=True,
                        )
                        nc.vector.scalar_tensor_tensor(
                            out=osb3[:, s0:s1, hd * HC : (hd + 1) * HC],
                            in0=hg3[0:98, s0:s1, hd * HC : (hd + 1) * HC],
                            scalar=dgv_sb[:, hd : hd + 1],
                            in1=po3[0:98, s0:s1],
                            op0=MUL,
                            op1=ADD,
                        )
                    if half == 0:
                        store(gi, osb3, 0, 4)
                    else:
                        store(gi, osb3, 4, 6)
                        store(gi, osb3, 6, 8)

            # ---- main emission ------------------------------------------
            # chunk k is fully transposed once pair t_need[k] is done
            t_need = [
                -(-(CHW * (k + 1) + OVL) // (2 * P)) for k in range(NCH)
            ]
            t_need[NCH - 1] = NPAIR
            # DMA groups needed to cover chunk k's transposes
            dg_need = [min(-(-(2 * t_need[k]) // DGF), NDG) for k in range(NCH)]

            # group-0 head: 4 pairs -> b0 -> 3 pairs.  Remaining head DMAs
            # (pairs 4-6, gm, bias rows) are interleaved after the first
            # two transposes so their descriptor-gen doesn't delay the
            # first routing copies.
            split_pair_T(0)
            split_pair_T(1)
            for pr in range(4, SPL):
                pair_dma(pr)
            nc.sync.dma_start(gm_sb[:], gmat[:])
            nc.sync.dma_start(hgt[0][98:99, :], brow[:])
            nc.sync.dma_start(hgt[1][98:99, :], brow[:])
            split_pair_T(2)
            split_pair_T(3)
            dma_groups = 1  # group 0 done via split path
            t_pairs = SPL
            prev = None
            for gi in range(NGRP):
                ck = gi // 2
                while dma_groups < dg_need[min((gi + 2) // 2, NCH - 1)]:
                    emit_a_dma(dma_groups)
                    dma_groups += 1
                if gi == 0:
                    emit_b_phase1(0, None)
                    for pr in range(4, SPL):
                        split_pair_T(pr)
                    prev = 0
                    continue
                while t_pairs < t_need[ck]:
                    emit_pair_T(t_pairs)
                    t_pairs += 1
                emit_b_phase1(gi, prev)
                prev = gi
            emit_p2_flush(prev)

    nc.compile()
    return nc


def _host_prep(W, e, bias, rows, cols):
    """Precompute fp16 device constants from the small parameter tensors."""
    W = np.asarray(W, np.float32)
    e = np.asarray(e, np.float32)
    bias = np.asarray(bias, np.float32)
    rows = np.asarray(rows, np.int64)
    cols = np.asarray(cols, np.int64)

    logits = np.full((HEADS, NLM, NLM), NEG, np.float64)
    logits[:, rows, cols] = e.astype(np.float64)
    m = logits.max(axis=-1, keepdims=True)
    p = np.exp(logits - m)
    A = p / p.sum(axis=-1, keepdims=True)            # [H, N, N]
    dg = np.einsum("hii->hi", A).copy()              # [H, N]
    A_off = A.copy()
    np.einsum("hii->hi", A_off)[:] = 0.0

    # Wall: [c, (s, h, d)] -> chunked [128, 2, 512]
    wr = W.transpose(2, 1, 0, 3).reshape(CIN, 2 * HEADS * HC)   # [c, shd]
    wall = np.ascontiguousarray(
        wr.reshape(2, P, 2 * HEADS * HC).transpose(1, 0, 2)
    ).astype(np.float16)

    # graph matrices: [j, (head, i)]; row 98 = all-ones bias row
    gm = np.zeros((P, HEADS, P), np.float32)
    for h in range(HEADS):
        gm[:NLM, h, :NLM] = A_off[h].T
        gm[NLM, h, :NLM] = 1.0
    gmat = np.ascontiguousarray(gm.reshape(P, HEADS * P)).astype(np.float16)

    dgvt = np.ascontiguousarray(dg.T).astype(np.float32)        # [98, 4]

    # bias row pattern for hgrp row 98: [s, (part, h, d)], part-1 = bias
    br = np.zeros((G, 2, HEADS * HC), np.float32)
    br[:, 1, :] = bias.reshape(HEADS * HC)
    brow = np.ascontiguousarray(br.reshape(1, G * HD)).astype(np.float16)

    ident = np.eye(P, dtype=np.float16)
    return {"wall": wall, "gmat": gmat, "dgvt": dgvt, "brow": brow,
            "ident": ident}


def kernel(x, W, e, bias, rows, cols):
    from concourse.bass_utils import run_bass_kernel_spmd

    if "nc" not in _CACHE:
        _CACHE["nc"] = _build_nc()
    nc = _CACHE["nc"]

    consts = _host_prep(W, e, bias, rows, cols)
    x = np.ascontiguousarray(np.asarray(x, np.float32)).reshape(B, NLM, CIN)

    in_maps = []
    for ci in range(NCORES):
        shard = np.ascontiguousarray(
            x[ci * NS : (ci + 1) * NS].reshape(NS * NLM, CIN)
        )
        in_maps.append({"x": shard, **consts})

    res = run_bass_kernel_spmd(
        nc,
        in_maps,
        core_ids=list(range(NCORES)),
        trace=bool(int(os.environ.get("KERNEL_TRACE", "0"))),
    )
    _CACHE["last_results"] = res

    out = np.concatenate(
        [r["out"].reshape(NS, NLM, HEADS * HC) for r in res.results], axis=0
    )
    return out


# revision 13
# speedup vs baseline: 1.2169x; 1.0534x over previous
"""MultiHeadSemGConv Trainium2 kernel.

Computes, for x:[B,N,CIN], W:[H,2,CIN,HC], e:[H,N*K], bias:[H,HC],
rows/cols:[N*K] (int32 edge list):

    h = einsum('bnc,hscd->shbnd', x, W)             # two projections per head
    A = softmax(scatter(e at (rows,cols), NEG))     # [H,N,N]
    out[h,b] = diag(A)*h0 + (A - diag)@h1 + bias    # -> [B,N,H*HC]

Strategy: pure data-parallel over batch across 8 NeuronCores.  The tiny
[H,98,98] adjacency softmax is precomputed on host; the heavy lifting
(x projection + graph mixing over 100MB of activations) runs on device:

  per core (128 samples):
    - DMA x in flat 128-partition tiles (group 0: f32 halves on the two
      HWDGE queues + DVE cast; later groups: fp16-casting SWDGE bulk)
    - PE transpose (matmul with identity) -> xT chunk tiles
      [c(2x128), 16*98+30 cols] fp16 in SBUF (30-col overlap keeps every
      per-sample phase-1 stationary at m=128)
    - phase 1, per sample b: h[128,512] = xT[:, 98b:98b+128].T @ Wall
      (2 accumulating fp16 matmuls, f32 PSUM), 2 samples per PSUM tile;
      rows [:98] copied into one of two persistent h tiles whose row 98
      holds the bias pattern
    - phase 2, per 8-sample group, per head: ONE matmul with the
      host-built A_off^T (contract k=99: 98 nodes + bias row), then a
      fused DVE op  out = dg (.) h0 + psum  adds the diagonal part.
      Phase 2 of group g is interleaved into phase 1 of group g+1.
    - DMA out f32
"""

import os
import sys

import numpy as np

try:
    import concourse.bass as bass  # noqa: F401
except Exception:  # pragma: no cover - fresh grading dir fallback
    for p in ("/opt/trn_rl_repo", "/root/.axon_site/_ro/trn_rl_repo"):
        if os.path.isdir(p) and p not in sys.path:
            sys.path.insert(0, p)
    import concourse.bass as bass  # noqa: F401

# ---------------------------------------------------------------- constants
NLM = 98          # landmarks (graph nodes)
HEADS = 4
CIN = 256
HC = 64
HD = 512          # h width = 2 (s) * 4 (heads) * 64 (d)
B = 1024
NCORES = 8
NS = B // NCORES  # samples per core = 128
P = 128
G = 8             # samples per output group
NGRP = NS // G    # 16 groups per core
OVL = 30          # overlap cols so every phase-1 lhsT can be m=128
NEG = -9e15

CHS = 16                    # samples per xT chunk
NCH = NS // CHS             # 8 chunks
CHW = CHS * NLM             # 1568 cols per chunk (+OVL)
NFT = NS * NLM // P         # 98 flat 128-row tiles
NPAIR = NFT // 2            # 49 transpose pairs
DGF = 14                    # flat tiles per input DMA group
NDG = NFT // DGF            # 7 DMA groups
SPL = 7                     # pairs in the split (head) group 0

_CACHE = {}


def _build_nc():
    import concourse.mybir as mybir
    import concourse.tile as tile
    from concourse import bacc

    f16 = mybir.dt.float16
    f32 = mybir.dt.float32
    MUL = mybir.AluOpType.mult
    ADD = mybir.AluOpType.add

    nc = bacc.Bacc(None, target_bir_lowering=False)

    x = nc.dram_tensor("x", [NS * NLM, CIN], f32, kind="ExternalInput")
    wall = nc.dram_tensor("wall", [P, 2, HD], f16, kind="ExternalInput")
    gmat = nc.dram_tensor("gmat", [P, HEADS * P], f16, kind="ExternalInput")
    dgvt = nc.dram_tensor("dgvt", [NLM, HEADS], f32, kind="ExternalInput")
    brow = nc.dram_tensor("brow", [1, G * HD], f16, kind="ExternalInput")
    ident = nc.dram_tensor("ident", [P, P], f16, kind="ExternalInput")
    out = nc.dram_tensor("out", [NS * NLM, CIN], f32, kind="ExternalOutput")

    with tile.TileContext(nc) as tc:
        with (
            tc.tile_pool(name="const", bufs=1) as constp,
            tc.tile_pool(name="x0", bufs=1) as x0p,
            tc.tile_pool(name="xf", bufs=2) as xfp,
            tc.tile_pool(name="xin", bufs=2) as xinp,
            tc.tile_pool(name="xt", bufs=1) as xtp,
            tc.tile_pool(name="hg", bufs=1) as hgp,
            tc.tile_pool(name="osb", bufs=2) as osbp,
            tc.tile_pool(name="ptr", bufs=2, space="PSUM") as ptrp,
            tc.tile_pool(name="phs", bufs=2, space="PSUM") as phsp,
            tc.tile_pool(name="pout", bufs=2, space="PSUM") as poutp,
        ):
            # ---- head DMAs: small consts + split group-0 x loads --------
            dgv_sb = constp.tile([NLM, HEADS], f32, tag="dgv")
            ident_sb = constp.tile([P, P], f16, tag="ident")
            nc.sync.dma_start(ident_sb[:], ident[:])

            xs32 = []
            for pr in range(SPL):
                xs = x0p.tile([P, 2, CIN], f32, tag=f"xs{pr}", name=f"xs{pr}")
                xs32.append(xs)
            wall_sb = constp.tile([P, 2, HD], f16, tag="wall")
            gm_sb = constp.tile([P, HEADS * P], f16, tag="gmat")

            def pair_dma(pr):
                # full 1KB-line DMA; alternate the two HWDGE queues so
                # descriptor-gen parallelizes while lines stay large
                b0 = pr * 2 * P
                src = x[b0 : b0 + 2 * P, :].rearrange("(t p) c -> p t c", p=P)
                eng = nc.sync if pr % 2 == 0 else nc.scalar
                eng.dma_start(xs32[pr][:], src)

            pair_dma(0)
            nc.scalar.dma_start(wall_sb[:], wall[:])
            for pr in range(1, 4):
                pair_dma(pr)

            hgt = [
                hgp.tile([P, G * HD], f16, tag=f"hg{i}", name=f"hg{i}")
                for i in range(2)
            ]

            xt = [
                xtp.tile([P, 2, CHW + OVL], f16, tag=f"xt{k}", name=f"xt{k}")
                for k in range(NCH)
            ]
            nc.vector.memset(xt[NCH - 1][:, :, CHW:], 0.0)

            def route_piece(g0, ptr, off, w):
                """Copy ptr[:, :, off:off+w] (global xT cols [g0,g0+w)) into
                the chunk tiles, including overlap duplication."""
                while w > 0:
                    k = g0 // CHW
                    lo = g0 - k * CHW
                    pw = min(w, CHW - lo)
                    nc.scalar.copy(
                        out=xt[k][:, :, lo : lo + pw],
                        in_=ptr[:, :, off : off + pw],
                    )
                    # overlap region of the previous chunk
                    if k > 0 and lo < OVL:
                        ow = min(pw, OVL - lo)
                        nc.scalar.copy(
                            out=xt[k - 1][:, :, CHW + lo : CHW + lo + ow],
                            in_=ptr[:, :, off : off + ow],
                        )
                    g0 += pw
                    off += pw
                    w -= pw

            def transpose_pair(xin_ap, gft):
                """Transpose 2 flat tiles (xin_ap: [P, 2, CIN] fp16) whose
                first global flat-tile index is gft."""
                ptr = ptrp.tile([P, 2, 2 * P], f32, tag="ptr")
                for a in range(2):
                    for cc in range(2):
                        nc.tensor.matmul(
                            ptr[:, cc, a * P : (a + 1) * P],
                            xin_ap[:, a, cc * P : (cc + 1) * P],
                            ident_sb[:],
                            start=True,
                            stop=True,
                        )
                route_piece(gft * P, ptr, 0, 2 * P)

            xf_t = {}

            def split_pair_T(pr):
                """Cast one head-group pair f32->fp16 on DVE, then PE
                transpose + route."""
                xf = xfp.tile([P, 2, CIN], f16, tag="xf")
                nc.vector.tensor_copy(xf[:], xs32[pr][:])
                transpose_pair(xf[:], pr * 2)
                xf_t[pr] = xf

            # ---- bulk input: fp16-casting SWDGE, DGF flat tiles a shot --
            # The SWDGE descriptor-gens would otherwise all fire at t=0 and
            # flood the DMA rings ahead of the latency-critical head loads,
            # so gens 1-2 get an artificial read-dependency on the head
            # casts; later gens are paced by the xin pool WAR (bufs=2).
            xin_tiles = {}
            scr = constp.tile([1, 8], f16, tag="scr")

            def emit_a_dma(dg):
                if dg == 1:
                    nc.gpsimd.tensor_copy(scr[0:1, 0:4], xf_t[1][0:1, 0, 0:4])
                elif dg == 2:
                    nc.gpsimd.tensor_copy(scr[0:1, 4:8], xf_t[3][0:1, 0, 0:4])
                xin = xinp.tile([P, DGF, CIN], f16, tag="xin")
                base = dg * DGF * P
                nc.gpsimd.dma_start(
                    xin[:],
                    x[base : base + DGF * P, :].rearrange("(t p) c -> p t c", p=P),
                )
                xin_tiles[dg] = xin

            def emit_pair_T(tp):
                """Transpose bulk pair tp (pairs SPL.. come from bulk)."""
                ft = tp * 2
                dg, pr = divmod(ft - 0, DGF)
                xin = xin_tiles[dg]
                transpose_pair(xin[:, pr : pr + 2, :], ft)

            hg3s = [h[:].rearrange("p (s f) -> p s f", s=G) for h in hgt]

            def emit_p2_head(gi, hd, osb3):
                """Phase 2 for one head of group gi: one k=99 matmul
                (A_off^T + bias row), then fused  out = dg (.) h0 + psum."""
                hg3 = hg3s[gi % 2]
                pouts = poutp.tile([P, G * HC], f32, tag="pout")
                po3 = pouts[:].rearrange("p (s f) -> p s f", s=G)
                nc.tensor.matmul(
                    po3,
                    gm_sb[0:99, hd * P : (hd + 1) * P],
                    hg3[0:99, :, 256 + hd * HC : 256 + (hd + 1) * HC],
                    start=True,
                    stop=True,
                )
                nc.vector.scalar_tensor_tensor(
                    out=osb3[:, :, hd * HC : (hd + 1) * HC],
                    in0=hg3[0:98, :, hd * HC : (hd + 1) * HC],
                    scalar=dgv_sb[:, hd : hd + 1],
                    in1=po3[0:98],
                    op0=MUL,
                    op1=ADD,
                )

            def store(gi, osb3, s0=0, s1=G):
                ov = out[gi * G * NLM : (gi + 1) * G * NLM, :].rearrange(
                    "(s i) c -> i s c", s=G
                )
                nc.sync.dma_start(ov[:, s0:s1], osb3[:, s0:s1])

            osb_t = {}

            def emit_b_phase1(gi, prev):
                """Phase 1 for G samples of gi; phase 2 of group `prev`
                interleaved between the pairs."""
                hgrp = hgt[gi % 2]
                if prev is not None:
                    osb = osbp.tile([NLM, G * 256], f32, tag="osb")
                    osb3 = osb[:].rearrange("p (s c) -> p s c", s=G)
                    osb_t[prev] = osb3
                ck = gi // 2
                for pi in range(G // 2):
                    hps = phsp.tile([P, 2, HD], f32, tag="hps")
                    for a in range(2):
                        b = gi * G + pi * 2 + a
                        lb = b - ck * CHS
                        for cc in range(2):
                            nc.tensor.matmul(
                                hps[:, a, :],
                                xt[ck][:, cc, NLM * lb : NLM * lb + P],
                                wall_sb[:, cc, :],
                                start=(cc == 0),
                                stop=(cc == 1),
                            )
                    dst = hgrp[0:98, pi * 2 * HD : (pi + 1) * 2 * HD].rearrange(
                        "p (a f) -> p a f", a=2
                    )
                    if pi == 0:
                        nc.vector.tensor_copy(dst, hps[0:98])
                    else:
                        nc.scalar.copy(out=dst, in_=hps[0:98])
                    if prev is not None:
                        if pi < 3:
                            emit_p2_head(prev, pi, osb_t[prev])
                        else:
                            emit_p2_head(prev, 3, osb_t[prev])
                            store(prev, osb_t[prev])

            def emit_p2_flush(gi):
                """Phase 2 for the final group, split in half-groups with
                split stores for a short kernel tail."""
                osb = osbp.tile([NLM, G * 256], f32, tag="osb")
                osb3 = osb[:].rearrange("p (s c) -> p s c", s=G)
                hg3 = hg3s[gi % 2]
                for half in range(2):
                    s0, s1 = half * 4, half * 4 + 4
                    for hd in range(HEADS):
                        pouts = poutp.tile([P, G * HC], f32, tag="pout")
                        po3 = pouts[:].rearrange("p (s f) -> p s f", s=G)
                        nc.tensor.matmul(
                            po3[:, s0:s1, :],
                            gm_sb[0:99, hd * P : (hd + 1) * P],
                            hg3[0:99, s0:s1, 256 + hd * HC : 256 + (hd + 1) * HC],
                            start=True,
                            stop=True,
                        )
                        nc.vector.scalar_tensor_tensor(
                            out=osb3[:, s0:s1, hd * HC : (hd + 1) * HC],
                            in0=hg3[0:98, s0:s1, hd * HC : (hd + 1) * HC],
                            scalar=dgv_sb[:, hd : hd + 1],
                            in1=po3[0:98, s0:s1],
                            op0=MUL,
                            op1=ADD,
                        )
                    if half == 0:
                        store(gi, osb3, 0, 4)
                    else:
                        store(gi, osb3, 4, 6)
                        store(gi, osb3, 6, 8)

            # ---- main emission ------------------------------------------
            # chunk k is fully transposed once pair t_need[k] is done
            t_need = [
                -(-(CHW * (k + 1) + OVL) // (2 * P)) for k in range(NCH)
            ]
            t_need[NCH - 1] = NPAIR
            # DMA groups needed to cover chunk k's transposes
            dg_need = [min(-(-(2 * t_need[k]) // DGF), NDG) for k in range(NCH)]

            # group-0 head: 4 pairs -> b0 -> 3 pairs.  Remaining head DMAs
            # (pairs 4-6, gm, bias rows) are interleaved after the first
            # two transposes so their descriptor-gen doesn't delay the
            # first routing copies.
            split_pair_T(0)
            split_pair_T(1)
            for pr in range(4, SPL):
                pair_dma(pr)
            nc.sync.dma_start(gm_sb[:], gmat[:])
            nc.scalar.dma_start(dgv_sb[:], dgvt[:])
            nc.sync.dma_start(hgt[0][98:99, :], brow[:])
            nc.sync.dma_start(hgt[1][98:99, :], brow[:])
            split_pair_T(2)
            split_pair_T(3)
            dma_groups = 1  # group 0 done via split path
            t_pairs = SPL
            prev = None
            for gi in range(NGRP):
                ck = gi // 2
                while dma_groups < dg_need[min((gi + 2) // 2, NCH - 1)]:
                    emit_a_dma(dma_groups)
                    dma_groups += 1
                if gi == 0:
                    emit_b_phase1(0, None)
                    for pr in range(4, SPL):
                        split_pair_T(pr)
                    prev = 0
                    continue
                while t_pairs < t_need[ck]:
                    emit_pair_T(t_pairs)
                    t_pairs += 1
                emit_b_phase1(gi, prev)
                prev = gi
            emit_p2_flush(prev)

    nc.compile()
    return nc


def _host_prep(W, e, bias, rows, cols):
    """Precompute fp16 device constants from the small parameter tensors."""
    W = np.asarray(W, np.float32)
    e = np.asarray(e, np.float32)
    bias = np.asarray(bias, np.float32)
    rows = np.asarray(rows, np.int64)
    cols = np.asarray(cols, np.int64)

    logits = np.full((HEADS, NLM, NLM), NEG, np.float64)
    logits[:, rows, cols] = e.astype(np.float64)
    m = logits.max(axis=-1, keepdims=True)
    p = np.exp(logits - m)
    A = p / p.sum(axis=-1, keepdims=True)            # [H, N, N]
    dg = np.einsum("hii->hi", A).copy()              # [H, N]
    A_off = A.copy()
    np.einsum("hii->hi", A_off)[:] = 0.0

    # Wall: [c, (s, h, d)] -> chunked [128, 2, 512]
    wr = W.transpose(2, 1, 0, 3).reshape(CIN, 2 * HEADS * HC)   # [c, shd]
    wall = np.ascontiguousarray(
        wr.reshape(2, P, 2 * HEADS * HC).transpose(1, 0, 2)
    ).astype(np.float16)

    # graph matrices: [j, (head, i)]; row 98 = all-ones bias row
    gm = np.zeros((P, HEADS, P), np.float32)
    for h in range(HEADS):
        gm[:NLM, h, :NLM] = A_off[h].T
        gm[NLM, h, :NLM] = 1.0
    gmat = np.ascontiguousarray(gm.reshape(P, HEADS * P)).astype(np.float16)

    dgvt = np.ascontiguousarray(dg.T).astype(np.float32)        # [98, 4]

    # bias row pattern for hgrp row 98: [s, (part, h, d)], part-1 = bias
    br = np.zeros((G, 2, HEADS * HC), np.float32)
    br[:, 1, :] = bias.reshape(HEADS * HC)
    brow = np.ascontiguousarray(br.reshape(1, G * HD)).astype(np.float16)

    ident = np.eye(P, dtype=np.float16)
    return {"wall": wall, "gmat": gmat, "dgvt": dgvt, "brow": brow,
            "ident": ident}


def kernel(x, W, e, bias, rows, cols):
    from concourse.bass_utils import run_bass_kernel_spmd

    if "nc" not in _CACHE:
        _CACHE["nc"] = _build_nc()
    nc = _CACHE["nc"]

    consts = _host_prep(W, e, bias, rows, cols)
    x = np.ascontiguousarray(np.asarray(x, np.float32)).reshape(B, NLM, CIN)

    in_maps = []
    for ci in range(NCORES):
        shard = np.ascontiguousarray(
            x[ci * NS : (ci + 1) * NS].reshape(NS * NLM, CIN)
        )
        in_maps.append({"x": shard, **consts})

    res = run_bass_kernel_spmd(
        nc,
        in_maps,
        core_ids=list(range(NCORES)),
        trace=bool(int(os.environ.get("KERNEL_TRACE", "0"))),
    )
    _CACHE["last_results"] = res

    out = np.concatenate(
        [r["out"].reshape(NS, NLM, HEADS * HC) for r in res.results], axis=0
    )
    return out


# revision 15
# speedup vs baseline: 1.2777x; 1.0499x over previous
"""MultiHeadSemGConv Trainium2 kernel.

Computes, for x:[B,N,CIN], W:[H,2,CIN,HC], e:[H,N*K], bias:[H,HC],
rows/cols:[N*K] (int32 edge list):

    h = einsum('bnc,hscd->shbnd', x, W)             # two projections per head
    A = softmax(scatter(e at (rows,cols), NEG))     # [H,N,N]
    out[h,b] = diag(A)*h0 + (A - diag)@h1 + bias    # -> [B,N,H*HC]

Strategy: pure data-parallel over batch across 8 NeuronCores.  The tiny
[H,98,98] adjacency softmax is precomputed on host; the heavy lifting
(x projection + graph mixing over 100MB of activations) runs on device:

  per core (128 samples):
    - DMA x in flat 128-partition tiles (group 0: f32 halves on the two
      HWDGE queues + DVE cast; later groups: fp16-casting SWDGE bulk)
    - PE transpose (matmul with identity) -> xT chunk tiles
      [c(2x128), 16*98+30 cols] fp16 in SBUF (30-col overlap keeps every
      per-sample phase-1 stationary at m=128)
    - phase 1, per sample b: h[128,512] = xT[:, 98b:98b+128].T @ Wall
      (2 accumulating fp16 matmuls, f32 PSUM), 2 samples per PSUM tile;
      rows [:98] copied into one of two persistent h tiles whose row 98
      holds the bias pattern
    - phase 2, per 8-sample group, per head: ONE matmul with the
      host-built A_off^T (contract k=99: 98 nodes + bias row), then a
      fused DVE op  out = dg (.) h0 + psum  adds the diagonal part.
      Phase 2 of group g is interleaved into phase 1 of group g+1.
    - DMA out f32
"""

import os
import sys

import numpy as np

try:
    import concourse.bass as bass  # noqa: F401
except Exception:  # pragma: no cover - fresh grading dir fallback
    for p in ("/opt/trn_rl_repo", "/root/.axon_site/_ro/trn_rl_repo"):
        if os.path.isdir(p) and p not in sys.path:
            sys.path.insert(0, p)
    import concourse.bass as bass  # noqa: F401

# ---------------------------------------------------------------- constants
NLM = 98          # landmarks (graph nodes)
HEADS = 4
CIN = 256
HC = 64
HD = 512          # h width = 2 (s) * 4 (heads) * 64 (d)
B = 1024
NCORES = 8
NS = B // NCORES  # samples per core = 128
P = 128
G = 8             # samples per output group
NGRP = NS // G    # 16 groups per core
OVL = 30          # overlap cols so every phase-1 lhsT can be m=128
NEG = -9e15

CHS = 16                    # samples per xT chunk
NCH = NS // CHS             # 8 chunks
CHW = CHS * NLM             # 1568 cols per chunk (+OVL)
NFT = NS * NLM // P         # 98 flat 128-row tiles
NPAIR = NFT // 2            # 49 transpose pairs
DGF = 14                    # flat tiles per input DMA group
NDG = NFT // DGF            # 7 DMA groups
SPL = 7                     # pairs in the split (head) group 0

_CACHE = {}


def _build_nc():
    import concourse.mybir as mybir
    import concourse.tile as tile
    from concourse import bacc

    f16 = mybir.dt.float16
    f32 = mybir.dt.float32
    MUL = mybir.AluOpType.mult
    ADD = mybir.AluOpType.add

    nc = bacc.Bacc(None, target_bir_lowering=False)

    x = nc.dram_tensor("x", [NS * NLM, CIN], f32, kind="ExternalInput")
    wall = nc.dram_tensor("wall", [P, 2, HD], f16, kind="ExternalInput")
    gmat = nc.dram_tensor("gmat", [P, HEADS * P], f16, kind="ExternalInput")
    dgvt = nc.dram_tensor("dgvt", [NLM, HEADS], f32, kind="ExternalInput")
    brow = nc.dram_tensor("brow", [1, G * HD], f16, kind="ExternalInput")
    ident = nc.dram_tensor("ident", [P, P], f16, kind="ExternalInput")
    out = nc.dram_tensor("out", [NS * NLM, CIN], f32, kind="ExternalOutput")

    with tile.TileContext(nc) as tc:
        with (
            tc.tile_pool(name="const", bufs=1) as constp,
            tc.tile_pool(name="x0", bufs=1) as x0p,
            tc.tile_pool(name="xf", bufs=2) as xfp,
            tc.tile_pool(name="xin", bufs=3) as xinp,
            tc.tile_pool(name="xt", bufs=1) as xtp,
            tc.tile_pool(name="hg", bufs=1) as hgp,
            tc.tile_pool(name="osb", bufs=2) as osbp,
            tc.tile_pool(name="ptr", bufs=2, space="PSUM") as ptrp,
            tc.tile_pool(name="phs", bufs=2, space="PSUM") as phsp,
            tc.tile_pool(name="pout", bufs=2, space="PSUM") as poutp,
        ):
            # ---- head DMAs: small consts + split group-0 x loads --------
            dgv_sb = constp.tile([NLM, HEADS], f32, tag="dgv")
            ident_sb = constp.tile([P, P], f16, tag="ident")
            nc.sync.dma_start(ident_sb[:], ident[:])

            xs32 = []
            for pr in range(SPL):
                xs = x0p.tile([P, 2, CIN], f32, tag=f"xs{pr}", name=f"xs{pr}")
                xs32.append(xs)
            wall_sb = constp.tile([P, 2, HD], f16, tag="wall")
            gm_sb = constp.tile([P, HEADS * P], f16, tag="gmat")

            def pair_dma(pr):
                # full 1KB-line DMA; alternate the two HWDGE queues so
                # descriptor-gen parallelizes while lines stay large
                b0 = pr * 2 * P
                src = x[b0 : b0 + 2 * P, :].rearrange("(t p) c -> p t c", p=P)
                eng = nc.sync if pr % 2 == 0 else nc.scalar
                eng.dma_start(xs32[pr][:], src)

            pair_dma(0)
            nc.scalar.dma_start(wall_sb[:], wall[:])
            for pr in range(1, 4):
                pair_dma(pr)

            hgt = [
                hgp.tile([P, G * HD], f16, tag=f"hg{i}", name=f"hg{i}")
                for i in range(2)
            ]

            xt = [
                xtp.tile([P, 2, CHW + OVL], f16, tag=f"xt{k}", name=f"xt{k}")
                for k in range(NCH)
            ]
            nc.vector.memset(xt[NCH - 1][:, :, CHW:], 0.0)

            def route_piece(g0, ptr, off, w):
                """Copy ptr[:, :, off:off+w] (global xT cols [g0,g0+w)) into
                the chunk tiles, including overlap duplication."""
                while w > 0:
                    k = g0 // CHW
                    lo = g0 - k * CHW
                    pw = min(w, CHW - lo)
                    nc.scalar.copy(
                        out=xt[k][:, :, lo : lo + pw],
                        in_=ptr[:, :, off : off + pw],
                    )
                    # overlap region of the previous chunk
                    if k > 0 and lo < OVL:
                        ow = min(pw, OVL - lo)
                        nc.scalar.copy(
                            out=xt[k - 1][:, :, CHW + lo : CHW + lo + ow],
                            in_=ptr[:, :, off : off + ow],
                        )
                    g0 += pw
                    off += pw
                    w -= pw

            def transpose_pair(xin_ap, gft):
                """Transpose 2 flat tiles (xin_ap: [P, 2, CIN] fp16) whose
                first global flat-tile index is gft."""
                ptr = ptrp.tile([P, 2, 2 * P], f32, tag="ptr")
                for a in range(2):
                    for cc in range(2):
                        nc.tensor.matmul(
                            ptr[:, cc, a * P : (a + 1) * P],
                            xin_ap[:, a, cc * P : (cc + 1) * P],
                            ident_sb[:],
                            start=True,
                            stop=True,
                        )
                route_piece(gft * P, ptr, 0, 2 * P)

            xf_t = {}

            def split_pair_T(pr):
                """Cast one head-group pair f32->fp16 on DVE, then PE
                transpose + route."""
                xf = xfp.tile([P, 2, CIN], f16, tag="xf")
                nc.vector.tensor_copy(xf[:], xs32[pr][:])
                transpose_pair(xf[:], pr * 2)
                xf_t[pr] = xf

            # ---- bulk input: fp16-casting SWDGE, DGF flat tiles a shot --
            # The SWDGE descriptor-gens would otherwise all fire at t=0 and
            # flood the DMA rings ahead of the latency-critical head loads.
            # The Tile scheduler reorders by data deps, so gens 1-2 are
            # gated with a WAW dep: a tiny copy (itself gated on a head
            # cast) into the DMA's own destination tile.  Later gens are
            # paced by the xin pool WAR (bufs=3).
            xin_tiles = {}

            def emit_a_dma(dg):
                xin = xinp.tile([P, DGF, CIN], f16, tag="xin")
                if dg == 1:
                    nc.gpsimd.tensor_copy(xin[0:1, 0, 0:4], xf_t[1][0:1, 0, 0:4])
                elif dg == 2:
                    nc.gpsimd.tensor_copy(xin[0:1, 0, 0:4], xf_t[3][0:1, 0, 0:4])
                base = dg * DGF * P
                nc.gpsimd.dma_start(
                    xin[:],
                    x[base : base + DGF * P, :].rearrange("(t p) c -> p t c", p=P),
                )
                xin_tiles[dg] = xin

            def emit_pair_T(tp):
                """Transpose bulk pair tp (pairs SPL.. come from bulk)."""
                ft = tp * 2
                dg, pr = divmod(ft - 0, DGF)
                xin = xin_tiles[dg]
                transpose_pair(xin[:, pr : pr + 2, :], ft)

            hg3s = [h[:].rearrange("p (s f) -> p s f", s=G) for h in hgt]

            def emit_p2_head(gi, hd, osb3):
                """Phase 2 for one head of group gi: one k=99 matmul
                (A_off^T + bias row), then fused  out = dg (.) h0 + psum."""
                hg3 = hg3s[gi % 2]
                pouts = poutp.tile([P, G * HC], f32, tag="pout")
                po3 = pouts[:].rearrange("p (s f) -> p s f", s=G)
                nc.tensor.matmul(
                    po3,
                    gm_sb[0:99, hd * P : (hd + 1) * P],
                    hg3[0:99, :, 256 + hd * HC : 256 + (hd + 1) * HC],
                    start=True,
                    stop=True,
                )
                nc.vector.scalar_tensor_tensor(
                    out=osb3[:, :, hd * HC : (hd + 1) * HC],
                    in0=hg3[0:98, :, hd * HC : (hd + 1) * HC],
                    scalar=dgv_sb[:, hd : hd + 1],
                    in1=po3[0:98],
                    op0=MUL,
                    op1=ADD,
                )

            def store(gi, osb3, s0=0, s1=G):
                ov = out[gi * G * NLM : (gi + 1) * G * NLM, :].rearrange(
                    "(s i) c -> i s c", s=G
                )
                nc.sync.dma_start(ov[:, s0:s1], osb3[:, s0:s1])

            osb_t = {}

            def emit_b_phase1(gi, prev):
                """Phase 1 for G samples of gi; phase 2 of group `prev`
                interleaved between the pairs."""
                hgrp = hgt[gi % 2]
                if prev is not None:
                    osb = osbp.tile([NLM, G * 256], f32, tag="osb")
                    osb3 = osb[:].rearrange("p (s c) -> p s c", s=G)
                    osb_t[prev] = osb3
                ck = gi // 2
                for pi in range(G // 2):
                    hps = phsp.tile([P, 2, HD], f32, tag="hps")
                    for a in range(2):
                        b = gi * G + pi * 2 + a
                        lb = b - ck * CHS
                        for cc in range(2):
                            nc.tensor.matmul(
                                hps[:, a, :],
                                xt[ck][:, cc, NLM * lb : NLM * lb + P],
                                wall_sb[:, cc, :],
                                start=(cc == 0),
                                stop=(cc == 1),
                            )
                    dst = hgrp[0:98, pi * 2 * HD : (pi + 1) * 2 * HD].rearrange(
                        "p (a f) -> p a f", a=2
                    )
                    if pi == 0:
                        nc.vector.tensor_copy(dst, hps[0:98])
                    else:
                        nc.scalar.copy(out=dst, in_=hps[0:98])
                    if prev is not None:
                        if pi < 3:
                            emit_p2_head(prev, pi, osb_t[prev])
                        else:
                            emit_p2_head(prev, 3, osb_t[prev])
                            store(prev, osb_t[prev])

            def emit_p2_flush(gi):
                """Phase 2 for the final group, split in half-groups with
                split stores for a short kernel tail."""
                osb = osbp.tile([NLM, G * 256], f32, tag="osb")
                osb3 = osb[:].rearrange("p (s c) -> p s c", s=G)
                hg3 = hg3s[gi % 2]
                for half in range(2):
                    s0, s1 = half * 4, half * 4 + 4
                    for hd in range(HEADS):
                        pouts = poutp.tile([P, G * HC], f32, tag="pout")
                        po3 = pouts[:].rearrange("p (s f) -> p s f", s=G)
                        nc.tensor.matmul(
                            po3[:, s0:s1, :],
                            gm_sb[0:99, hd * P : (hd + 1) * P],
                            hg3[0:99, s0:s1, 256 + hd * HC : 256 + (hd + 1) * HC],
                            start=True,
                            stop=True,
                        )
                        nc.vector.scalar_tensor_tensor(
                            out=osb3[:, s0:s1, hd * HC : (hd + 1) * HC],
                            in0=hg3[0:98, s0:s1, hd * HC : (hd + 1) * HC],
                            scalar=dgv_sb[:, hd : hd + 1],
                            in1=po3[0:98, s0:s1],
                            op0=MUL,
                            op1=ADD,
                        )
                    if half == 0:
                        store(gi, osb3, 0, 4)
                    else:
                        store(gi, osb3, 4, 6)
                        store(gi, osb3, 6, 8)

            # ---- main emission ------------------------------------------
            # chunk k is fully transposed once pair t_need[k] is done
            t_need = [
                -(-(CHW * (k + 1) + OVL) // (2 * P)) for k in range(NCH)
            ]
            t_need[NCH - 1] = NPAIR
            # DMA groups needed to cover chunk k's transposes
            dg_need = [min(-(-(2 * t_need[k]) // DGF), NDG) for k in range(NCH)]

            # group-0 head: 4 pairs -> b0 -> 3 pairs.  Remaining head DMAs
            # (pairs 4-6, gm, bias rows) are interleaved after the first
            # two transposes so their descriptor-gen doesn't delay the
            # first routing copies.
            split_pair_T(0)
            split_pair_T(1)
            for pr in range(4, SPL):
                pair_dma(pr)
            nc.sync.dma_start(gm_sb[:], gmat[:])
            nc.scalar.dma_start(dgv_sb[:], dgvt[:])
            nc.sync.dma_start(hgt[0][98:99, :], brow[:])
            nc.sync.dma_start(hgt[1][98:99, :], brow[:])
            split_pair_T(2)
            split_pair_T(3)
            dma_groups = 1  # group 0 done via split path
            t_pairs = SPL
            prev = None
            for gi in range(NGRP):
                ck = gi // 2
                while dma_groups < dg_need[min((gi + 2) // 2, NCH - 1)]:
                    emit_a_dma(dma_groups)
                    dma_groups += 1
                if gi == 0:
                    emit_b_phase1(0, None)
                    for pr in range(4, SPL):
                        split_pair_T(pr)
                    prev = 0
                    continue
                while t_pairs < t_need[ck]:
                    emit_pair_T(t_pairs)
                    t_pairs += 1
                emit_b_phase1(gi, prev)
                prev = gi
            emit_p2_flush(prev)

    nc.compile()
    return nc


def _host_prep(W, e, bias, rows, cols):
    """Precompute fp16 device constants from the small parameter tensors."""
    W = np.asarray(W, np.float32)
    e = np.asarray(e, np.float32)
    bias = np.asarray(bias, np.float32)
    rows = np.asarray(rows, np.int64)
    cols = np.asarray(cols, np.int64)

    logits = np.full((HEADS, NLM, NLM), NEG, np.float64)
    logits[:, rows, cols] = e.astype(np.float64)
    m = logits.max(axis=-1, keepdims=True)
    p = np.exp(logits - m)
    A = p / p.sum(axis=-1, keepdims=True)            # [H, N, N]
    dg = np.einsum("hii->hi", A).copy()              # [H, N]
    A_off = A.copy()
    np.einsum("hii->hi", A_off)[:] = 0.0

    # Wall: [c, (s, h, d)] -> chunked [128, 2, 512]
    wr = W.transpose(2, 1, 0, 3).reshape(CIN, 2 * HEADS * HC)   # [c, shd]
    wall = np.ascontiguousarray(
        wr.reshape(2, P, 2 * HEADS * HC).transpose(1, 0, 2)
    ).astype(np.float16)

    # graph matrices: [j, (head, i)]; row 98 = all-ones bias row
    gm = np.zeros((P, HEADS, P), np.float32)
    for h in range(HEADS):
        gm[:NLM, h, :NLM] = A_off[h].T
        gm[NLM, h, :NLM] = 1.0
    gmat = np.ascontiguousarray(gm.reshape(P, HEADS * P)).astype(np.float16)

    dgvt = np.ascontiguousarray(dg.T).astype(np.float32)        # [98, 4]

    # bias row pattern for hgrp row 98: [s, (part, h, d)], part-1 = bias
    br = np.zeros((G, 2, HEADS * HC), np.float32)
    br[:, 1, :] = bias.reshape(HEADS * HC)
    brow = np.ascontiguousarray(br.reshape(1, G * HD)).astype(np.float16)

    ident = np.eye(P, dtype=np.float16)
    return {"wall": wall, "gmat": gmat, "dgvt": dgvt, "brow": brow,
            "ident": ident}


def kernel(x, W, e, bias, rows, cols):
    from concourse.bass_utils import run_bass_kernel_spmd

    if "nc" not in _CACHE:
        _CACHE["nc"] = _build_nc()
    nc = _CACHE["nc"]

    consts = _host_prep(W, e, bias, rows, cols)
    x = np.ascontiguousarray(np.asarray(x, np.float32)).reshape(B, NLM, CIN)

    in_maps = []
    for ci in range(NCORES):
        shard = np.ascontiguousarray(
            x[ci * NS : (ci + 1) * NS].reshape(NS * NLM, CIN)
        )
        in_maps.append({"x": shard, **consts})

    res = run_bass_kernel_spmd(
        nc,
        in_maps,
        core_ids=list(range(NCORES)),
        trace=bool(int(os.environ.get("KERNEL_TRACE", "0"))),
    )
    _CACHE["last_results"] = res

    out = np.concatenate(
        [r["out"].reshape(NS, NLM, HEADS * HC) for r in res.results], axis=0
    )
    return out
